# revision 1
# baseline (speedup 1.0000x reference)
"""Trainium2 Bass kernel for nn_Attention (non-local-block attention).

Reference (per batch b, z flattened to [Ci=128, N=4096]):
    theta = w_theta @ z + b_theta        [64, N]
    phi   = w_phi   @ z + b_phi          [64, N]
    psi   = w_psi   @ z + b_psi          [64, N]
    G     = relu((phi^T psi) / N)        [N, N]
    out   = w_v @ (G @ theta^T)^T + b_v + z

Restructure: fold w_v into theta (host: WTH = w_v @ w_theta, bvth = w_v @
b_theta), so WVTH = WTH @ z + bvth and
    out[ci,n] = (1/N) * sum_m relu(g)[n,m] * WVTH[ci,m] + b_v[ci] + z[ci,n]
which turns the second attention matmul into the final 128-channel output
directly (no separate w_v tail) with full K=128/M=128 array use.

Sharding: 8 cores = 2 batches x 4 token-blocks of 1024. Per core the m
(token) dim is processed in 4 chunks of 1024: chunk 0 is the core's own
block (bf16, needed exactly for phi + residual), chunks 1-3 are the other
blocks shipped as fp8e4 (the attention path tolerates it; input DMA bytes
drop 1MB -> 0.64MB at the measured ~120GB/s input rate). z8 rides the
gpsimd DMA queue in parallel with the sync queue, and chunks are processed
in arrival order (1,2,3,0).

Per-core dataflow (m-tile = 128 tokens, 32 m-tiles):
  24 dense 128-col warmup matmuls (~3.4us) fill one HAM SHORT window so the
      PE clock ramps 1.2 -> 2.4 GHz; shorter bursts measurably fail to
      ignite and the whole kernel then runs at half clock.
  psi/phi projections with row-duplicated weights: even m-tiles use
      partitions 0:64, odd use 64:128, so consecutive g matmuls sit on
      different PE row-groups and LDWEIGHTS overlaps in-flight matmuls.
  wvthT per m-tile [128m, 128ci] fp8: z-chunk-stationary matmuls onto a
      K=1 ones x bvth-tiled PSUM prefill (bias, no broadcast-tile DMA);
      drains alternate ScalarE/VectorE and interleave with the pair loop.
  vg PSUM [128,1024] f32 = 4096*z prefill (identity matmul; residual +
      1/4096 scale + b_v applied at the final drain), accumulating
      sum_m s * wvthT via fp8 DoubleRow (2 m-tiles per pass, lag 2 pairs).
  relu: even m-tile on ScalarE, odd on VectorE (2 pairs run both on
      ScalarE for engine balance), fp8 out, 3-slot PSUM ring.
  tail: out = vg/4096 + b_v split ScalarE/VectorE, 2 DMAs.

Measured (core-0 NTFF span): 59.7us on a clean-clock run (vs 63.4us
baseline); throttled-chip runs land ~78us with every engine ~20% slower.
"""

import ml_dtypes
import numpy as np

import concourse.bacc as bacc
import concourse.mybir as mybir
import concourse.tile as tile
from concourse.bass_utils import run_bass_kernel_spmd

F32 = mybir.dt.float32
BF16 = mybir.dt.bfloat16
FP8 = mybir.dt.float8e4
AF = mybir.ActivationFunctionType
ALU = mybir.AluOpType
DR = mybir.MatmulPerfMode.DoubleRow
BF16NP = ml_dtypes.bfloat16
FP8NP = ml_dtypes.float8_e4m3

B, CI, CO = 2, 128, 64
T, H, W = 4, 32, 32
N = T * H * W            # 4096 tokens
NCORES = 8
BLK = N // (NCORES // B)  # 1024 tokens per core
MT = N // 128            # 32 m-tiles
SCALE = float(1.0 / N)
CHUNK_ORDER = (1, 2, 3, 0)   # z8 chunks land before zblk16

_CACHE = {}


def _build():
    nc = bacc.Bacc("TRN2", target_bir_lowering=False, debug=False)

    wpack = nc.dram_tensor("wpack", [CI, 512], BF16, kind="ExternalInput")
    w8pack = nc.dram_tensor("w8pack", [CI, 256], FP8, kind="ExternalInput")
    biaspack = nc.dram_tensor("biaspack", [CI, 4], F32, kind="ExternalInput")
    zblk16 = nc.dram_tensor("zblk16", [CI, BLK], BF16, kind="ExternalInput")
    z8 = nc.dram_tensor("z8", [CI, 3 * BLK], FP8, kind="ExternalInput")
    smallpack = nc.dram_tensor("smallpack", [1, 640], BF16, kind="ExternalInput")
    out = nc.dram_tensor("out", [CI, BLK], F32, kind="ExternalOutput")

    with tile.TileContext(nc) as tc:
        with (
            tc.tile_pool(name="const", bufs=1) as cpool,
            tc.tile_pool(name="zp", bufs=1) as zp,
            tc.tile_pool(name="proj", bufs=1) as pp,
            tc.tile_pool(name="s8p", bufs=3) as sp,
            tc.tile_pool(name="tail", bufs=1) as tailp,
            tc.tile_pool(name="vgps", bufs=1, space="PSUM") as vgpool,
        ):
            # ---- input DMAs: sync queue for bf16 path, gpsimd for fp8 ----
            wpack_sb = cpool.tile([CI, 512], BF16)
            nc.sync.dma_start(wpack_sb[:], wpack[:])
            w8pack_sb = cpool.tile([CI, 256], FP8)
            nc.sync.dma_start(w8pack_sb[:], w8pack[:])
            biaspack_sb = cpool.tile([CI, 4], F32)
            nc.sync.dma_start(biaspack_sb[:], biaspack[:])
            smallpack_sb = cpool.tile([1, 640], BF16)
            nc.sync.dma_start(smallpack_sb[:], smallpack[:])
            zblk16_sb = zp.tile([CI, BLK], BF16)
            nc.sync.dma_start(zblk16_sb[:], zblk16[:])
            z8_sb = zp.tile([CI, 3 * BLK], FP8)
            for c in range(3):
                nc.gpsimd.dma_start(
                    z8_sb[:, c * BLK:(c + 1) * BLK],
                    z8[:, c * BLK:(c + 1) * BLK],
                )

            wpsid_sb = wpack_sb[:, 0:128]       # [wpsiT | wpsiT]
            wphid_sb = wpack_sb[:, 128:256]     # [wphiT | wphiT]
            wtht_sb = wpack_sb[:, 256:384]      # (w_v @ w_theta).T bf16
            ident_sb = wpack_sb[:, 384:512]     # 4096 * I
            wpsid8_sb = w8pack_sb[:, 0:128]
            wtht8_sb = w8pack_sb[:, 128:256]
            bpsi_sb = biaspack_sb[:, 0:1]       # [b_psi; b_psi]
            bphi_sb = biaspack_sb[:, 2:3]       # [b_phi; b_phi]
            bv_sb = biaspack_sb[:, 1:2]
            ones_sb = smallpack_sb[:, 0:128]
            bvtht_sb = smallpack_sb[:, 128:640]

            psi_sb = pp.tile([CI, N], BF16)     # row-duplicated
            phi_sb = pp.tile([CI, BLK], BF16)   # row-duplicated
            wvth8_sb = pp.tile([CI, MT, CI], FP8)

            # vg accumulator: 2 PSUM banks, lives for the whole kernel
            vg_ps = vgpool.tile([CI, BLK], F32)

            # ---- HAM ignition: ~3.4us of dense 128-col matmuls on wpack
            # (arrives early) fills one SHORT window so the PE clock ramps
            # 1.2 -> 2.4 GHz before the DMA-paced projections begin. ----
            with tc.tile_pool(name="warm", bufs=1, space="PSUM") as wpool:
                wps = wpool.tile([CI, 512], F32)
                for _ in range(24):
                    nc.tensor.matmul(
                        wps[:, 0:128], wpsid_sb, wpsid_sb,
                        skip_group_check=True,
                    )

            s8 = {}
            g = {}
            vg_n = [0]

            def zslice(mt, width=128):
                c = mt // 8
                o = (mt % 8) * 128
                if c == 0:
                    return zblk16_sb[:, o:o + width]
                return z8_sb[:, (c - 1) * BLK + o:(c - 1) * BLK + o + width]

            def emit_g(mt, gpool):
                gt = gpool.tile([CI, 1024], F32, tag="g", name=f"g{mt}")
                hb = 0 if mt % 2 == 0 else CO   # row-group alternation
                msl = slice(mt * 128, (mt + 1) * 128)
                for h in range(2):
                    nc.tensor.matmul(
                        gt[:, h * 512:(h + 1) * 512],
                        psi_sb[hb:hb + CO, msl],
                        phi_sb[hb:hb + CO, h * 512:(h + 1) * 512],
                    )
                g[mt] = gt

            def emit_relu(p, a, b, odd_on_act=False):
                s = sp.tile([CI, 2, 1024], FP8, tag="s", name=f"s{p}")
                nc.scalar.activation(s[:, 0, :], g.pop(a)[:], AF.Relu)
                if odd_on_act:
                    nc.scalar.activation(s[:, 1, :], g.pop(b)[:], AF.Relu)
                else:
                    nc.vector.tensor_scalar_max(s[:, 1, :], g.pop(b)[:], 0.0)
                s8[p] = s

            def emit_vg(p, a):
                s = s8.pop(p)
                vg_n[0] += 1
                for h in range(2):
                    nc.tensor.matmul(
                        vg_ps[:, h * 512:(h + 1) * 512],
                        wvth8_sb[:, a:a + 2, :],
                        s[:, :, h * 512:(h + 1) * 512],
                        start=False, stop=(vg_n[0] == MT // 2),
                        perf_mode=DR, skip_group_check=True,
                    )

            def emit_psi(c, gpool):
                base = c * 1024
                ps = gpool.tile([CI, 1024], F32, tag="g", name=f"pj{c}")
                for h in range(2):
                    if c == 0:
                        nc.tensor.matmul(
                            ps[:, h * 512:(h + 1) * 512], wpsid_sb,
                            zblk16_sb[:, h * 512:(h + 1) * 512],
                        )
                    else:
                        nc.tensor.matmul(
                            ps[:, h * 512:(h + 1) * 512], wpsid8_sb,
                            z8_sb[:, (c - 1) * BLK + h * 512:
                                  (c - 1) * BLK + (h + 1) * 512],
                        )
                # split drain across both engines: halves the PSUM
                # slot-recycle latency in the (engine-idle) front
                nc.scalar.activation(
                    psi_sb[:, base:base + 512], ps[:, 0:512],
                    AF.Identity, bias=bpsi_sb,
                )
                nc.vector.tensor_scalar_add(
                    psi_sb[:, base + 512:base + 1024], ps[:, 512:1024],
                    bpsi_sb,
                )

            def emit_phi(gpool):
                php = gpool.tile([CI, 1024], F32, tag="g", name="phj")
                for h in range(2):
                    nc.tensor.matmul(
                        php[:, h * 512:(h + 1) * 512], wphid_sb,
                        zblk16_sb[:, h * 512:(h + 1) * 512],
                    )
                nc.scalar.activation(
                    phi_sb[:, 0:512], php[:, 0:512],
                    AF.Identity, bias=bphi_sb,
                )
                nc.vector.tensor_scalar_add(
                    phi_sb[:, 512:1024], php[:, 512:1024], bphi_sb
                )

            def emit_wvth(c, gpool, on_act):
                wv_ps = gpool.tile([CI, 1024], F32, tag="g", name=f"wv{c}")
                for h in range(2):
                    nc.tensor.matmul(
                        wv_ps[:, h * 512:(h + 1) * 512],
                        ones_sb, bvtht_sb,
                        start=True, stop=False, skip_group_check=True,
                    )
                for j in range(8):
                    mt = c * 8 + j
                    nc.tensor.matmul(
                        wv_ps[:, j * 128:(j + 1) * 128],
                        zslice(mt),
                        wtht_sb if c == 0 else wtht8_sb,
                        start=False, stop=(j % 4 == 3),
                        skip_group_check=True,
                    )
                dst = wvth8_sb[:, c * 8:(c + 1) * 8, :]
                if on_act:
                    nc.scalar.activation(dst, wv_ps[:], AF.Copy)
                else:
                    nc.vector.tensor_copy(dst, wv_ps[:])

            with tc.tile_pool(name="gps", bufs=3, space="PSUM") as gpool:
                # projections in z-arrival order; psi for all chunks first
                # (only needs z + weights), then phi, then wvth interleaved
                # with the first pairs.
                for c in CHUNK_ORDER:
                    emit_psi(c, gpool)
                emit_phi(gpool)
                # residual prefill: vg_ps = 4096 * z(block)
                for h in range(2):
                    nc.tensor.matmul(
                        vg_ps[:, h * 512:(h + 1) * 512],
                        ident_sb,
                        zblk16_sb[:, h * 512:(h + 1) * 512],
                        start=True, stop=False, skip_group_check=True,
                    )
                pairs = [4 * c + i for c in CHUNK_ORDER for i in range(4)]
                plan = []
                for k, p in enumerate(pairs):
                    if k % 4 == 0:
                        plan.append(("wvth", CHUNK_ORDER[k // 4]))
                    plan.append(("pair", p))
                done = []
                wvth_act = {2: True, 0: True, 1: False, 3: False}
                act_takeover = {pairs[1], pairs[8]}
                for kind, val in plan:
                    if kind == "wvth":
                        emit_wvth(val, gpool, wvth_act[val])
                    else:
                        emit_g(2 * val, gpool)
                        emit_g(2 * val + 1, gpool)
                        done.append(val)
                        if len(done) >= 3:
                            emit_vg(done[-3], 2 * done[-3])
                        emit_relu(val, 2 * val, 2 * val + 1,
                                  odd_on_act=val in act_takeover)
                emit_vg(done[-2], 2 * done[-2])
                emit_vg(done[-1], 2 * done[-1])

            # ---- tail: out = vg/4096 + b_v ----
            out0 = tailp.tile([CI, 512], F32, name="out0")
            out1 = tailp.tile([CI, 512], F32, name="out1")
            nc.scalar.activation(
                out0[:], vg_ps[:, 0:512], AF.Identity, bias=bv_sb, scale=SCALE
            )
            nc.vector.tensor_scalar(
                out1[:], vg_ps[:, 512:1024], SCALE, bv_sb, ALU.mult, ALU.add
            )
            nc.sync.dma_start(out[:, 0:512], out0[:])
            nc.sync.dma_start(out[:, 512:1024], out1[:])

    nc.compile()
    return nc


def _get_nc():
    if "nc" not in _CACHE:
        _CACHE["nc"] = _build()
    return _CACHE["nc"]


def build_in_maps(z, w_theta, b_theta, w_phi, b_phi, w_psi, b_psi, w_v, b_v):
    z2 = np.asarray(z, np.float32).reshape(B, CI, N)

    wpsiT = np.asarray(w_psi, np.float32).T          # [128, 64]
    wphiT = np.asarray(w_phi, np.float32).T
    wv = np.asarray(w_v, np.float32)                 # [128, 64]
    wthT = (wv @ np.asarray(w_theta, np.float32)).T  # [128, 128]
    wpack = np.ascontiguousarray(
        np.concatenate(
            [wpsiT, wpsiT, wphiT, wphiT, wthT,
             np.float32(N) * np.eye(CI, dtype=np.float32)],
            axis=1,
        ).astype(BF16NP)
    )
    w8pack = np.ascontiguousarray(
        np.concatenate([wpsiT, wpsiT, wthT], axis=1).astype(FP8NP)
    )
    biaspack = np.stack(
        [
            np.concatenate([b_psi, b_psi]).astype(np.float32),
            np.asarray(b_v, np.float32),
            np.concatenate([b_phi, b_phi]).astype(np.float32),
            np.zeros(CI, np.float32),
        ],
        axis=1,
    ).astype(np.float32)
    bvth = wv @ np.asarray(b_theta, np.float32)      # [128]
    smallpack = np.concatenate(
        [np.ones(128, np.float32), np.tile(bvth, 4)]
    ).reshape(1, 640).astype(BF16NP)

    in_maps = []
    for core in range(NCORES):
        b, nb = divmod(core, NCORES // B)
        others = [c for c in range(4) if c != nb]
        zrest = np.concatenate(
            [z2[b][:, c * BLK:(c + 1) * BLK] for c in others], axis=1
        )
        in_maps.append(
            {
                "wpack": wpack,
                "w8pack": w8pack,
                "biaspack": biaspack,
                "smallpack": smallpack,
                "zblk16": np.ascontiguousarray(
                    z2[b][:, nb * BLK:(nb + 1) * BLK].astype(BF16NP)
                ),
                "z8": np.ascontiguousarray(zrest.astype(FP8NP)),
            }
        )
    return in_maps


def kernel(z, w_theta, b_theta, w_phi, b_phi, w_psi, b_psi, w_v, b_v):
    in_maps = build_in_maps(
        z, w_theta, b_theta, w_phi, b_phi, w_psi, b_psi, w_v, b_v
    )
    nc = _get_nc()
    res = run_bass_kernel_spmd(nc, in_maps, core_ids=list(range(NCORES)))

    out_full = np.empty((B, CI, N), dtype=np.float32)
    for core in range(NCORES):
        b, nb = divmod(core, NCORES // B)
        out_full[b][:, nb * BLK:(nb + 1) * BLK] = res.results[core]["out"]
    return out_full.reshape(B, CI, T, H, W)



# revision 3
# speedup vs baseline: 1.0492x; 1.0492x over previous
"""Trainium2 Bass kernel for nn_Attention (non-local-block attention).

Reference (per batch b, z flattened to [Ci=128, N=4096]):
    theta = w_theta @ z + b_theta        [64, N]
    phi   = w_phi   @ z + b_phi          [64, N]
    psi   = w_psi   @ z + b_psi          [64, N]
    G     = relu((phi^T psi) / N)        [N, N]
    out   = w_v @ (G @ theta^T)^T + b_v + z

Restructure: fold w_v into theta (host: WTH = w_v @ w_theta, bvth = w_v @
b_theta), so WVTH = WTH @ z + bvth and
    out[ci,n] = (1/N) * sum_m relu(g)[n,m] * WVTH[ci,m] + b_v[ci] + z[ci,n]

Sharding: 8 cores = 2 batches x 4 token-blocks of 1024. Chunks 1-3 of the
m (token) dim ship as fp8e4; the core's own block as bf16.

v2 changes (driven by the p-state model: PE reaches 2.4GHz only after 3us
of CONTINUOUS execution; every stall resets the ramp to 1.2GHz):
  - warmup matmuls source a gpsimd-memset tile instead of DMA'd weights,
    so the PE starts ~3us earlier (right after the preamble) and is
    already ramped when the first z chunk lands.
  - the 4096 cols of K=1 ones x bvth bias-prefill matmuls are gone:
    bvth is broadcast across partitions once (one 128-col matmul) and
    added during the wvth PSUM drain via stride-0 tensor_tensor on
    Pool/DVE (ACT cannot do free-dim bias).
  - wvth drains moved off ACT/DVE's critical relu path: halves on
    Pool + DVE, with the next pair's relu taken over by ACT.
  - ACT act-table load hoisted to kernel start via a dummy activation
    (it cost 1.5us right before the first psi drain).
  - input DMA order: brow, w8pack, biaspack, zblk16, wpack on sync
    queue (need-order), z8 chunks on the gpsimd queue.
"""

import ml_dtypes
import numpy as np

import concourse.bacc as bacc
import concourse.mybir as mybir
import concourse.tile as tile
from concourse.bass_utils import run_bass_kernel_spmd

F32 = mybir.dt.float32
BF16 = mybir.dt.bfloat16
FP8 = mybir.dt.float8e4
AF = mybir.ActivationFunctionType
ALU = mybir.AluOpType
DR = mybir.MatmulPerfMode.DoubleRow
BF16NP = ml_dtypes.bfloat16
FP8NP = ml_dtypes.float8_e4m3

B, CI, CO = 2, 128, 64
T, H, W = 4, 32, 32
N = T * H * W            # 4096 tokens
NCORES = 8
BLK = N // (NCORES // B)  # 1024 tokens per core
MT = N // 128            # 32 m-tiles
SCALE = float(1.0 / N)
CHUNK_ORDER = (1, 2, 3, 0)   # z8 chunks land before zblk16
N_WARM = 26

_CACHE = {}


def _build():
    nc = bacc.Bacc("TRN2", target_bir_lowering=False, debug=False)

    wpack = nc.dram_tensor("wpack", [CI, 512], BF16, kind="ExternalInput")
    w8pack = nc.dram_tensor("w8pack", [CI, 256], FP8, kind="ExternalInput")
    biaspack = nc.dram_tensor("biaspack", [CI, 4], F32, kind="ExternalInput")
    brow = nc.dram_tensor("brow", [1, 128], BF16, kind="ExternalInput")
    zblk16 = nc.dram_tensor("zblk16", [CI, BLK], BF16, kind="ExternalInput")
    z8 = nc.dram_tensor("z8", [CI, 3 * BLK], FP8, kind="ExternalInput")
    out = nc.dram_tensor("out", [CI, BLK], F32, kind="ExternalOutput")

    with tile.TileContext(nc) as tc:
        with (
            tc.tile_pool(name="const", bufs=1) as cpool,
            tc.tile_pool(name="zp", bufs=1) as zp,
            tc.tile_pool(name="proj", bufs=1) as pp,
            tc.tile_pool(name="s8p", bufs=3) as sp,
            tc.tile_pool(name="tail", bufs=1) as tailp,
            tc.tile_pool(name="vgps", bufs=1, space="PSUM") as vgpool,
        ):
            # ---- warm sources: no DMA dependency ----
            warmsrc = cpool.tile([CI, 128], BF16)
            nc.gpsimd.memset(warmsrc[:], 1.0)
            ones1 = cpool.tile([1, 128], BF16)
            nc.gpsimd.memset(ones1[:], 1.0)
            actdummy = cpool.tile([CI, 2], F32)
            nc.gpsimd.memset(actdummy[:], 0.0)

            # ---- input DMAs in need-order ----
            brow_sb = cpool.tile([1, 128], BF16)
            nc.sync.dma_start(brow_sb[:], brow[:])
            w8pack_sb = cpool.tile([CI, 256], FP8)
            nc.sync.dma_start(w8pack_sb[:], w8pack[:])
            biaspack_sb = cpool.tile([CI, 4], F32)
            nc.sync.dma_start(biaspack_sb[:], biaspack[:])
            zblk16_sb = zp.tile([CI, BLK], BF16)
            nc.sync.dma_start(zblk16_sb[:], zblk16[:])
            wpack_sb = cpool.tile([CI, 512], BF16)
            nc.sync.dma_start(wpack_sb[:], wpack[:])
            z8_sb = zp.tile([CI, 3 * BLK], FP8)
            for c in range(3):
                nc.gpsimd.dma_start(
                    z8_sb[:, c * BLK:(c + 1) * BLK],
                    z8[:, c * BLK:(c + 1) * BLK],
                )

            # ---- ACT table load hoist (1.5us otherwise paid at first
            # psi drain) ----
            actout = cpool.tile([CI, 2], F32)
            nc.scalar.activation(actout[:], actdummy[:], AF.Relu)

            wpsid_sb = wpack_sb[:, 0:128]       # [wpsiT | wpsiT]
            wphid_sb = wpack_sb[:, 128:256]     # [wphiT | wphiT]
            wtht_sb = wpack_sb[:, 256:384]      # (w_v @ w_theta).T bf16
            ident_sb = wpack_sb[:, 384:512]     # 4096 * I
            wpsid8_sb = w8pack_sb[:, 0:128]
            wtht8_sb = w8pack_sb[:, 128:256]
            bpsi_sb = biaspack_sb[:, 0:1]       # [b_psi; b_psi]
            bphi_sb = biaspack_sb[:, 2:3]       # [b_phi; b_phi]
            bv_sb = biaspack_sb[:, 1:2]

            psi_sb = pp.tile([CI, N], BF16)     # row-duplicated
            phi_sb = pp.tile([CI, BLK], BF16)   # row-duplicated
            wvth8_sb = pp.tile([CI, MT, CI], FP8)
            bvthrow_sb = pp.tile([CI, 1, CI], F32)

            # vg accumulator: 2 PSUM banks, lives for the whole kernel
            vg_ps = vgpool.tile([CI, BLK], F32)

            # ---- warmup: PE busy from the moment the preamble ends, so
            # the p-state ramp (3us continuous -> 2.4GHz) completes right
            # as the first z chunk lands. Also: bvth broadcast across
            # partitions via a K=1 ones matmul. ----
            with tc.tile_pool(name="warm", bufs=1, space="PSUM") as wpool:
                wps = wpool.tile([CI, 512], F32)
                for _ in range(N_WARM):
                    nc.tensor.matmul(
                        wps[:, 0:128], warmsrc[:], warmsrc[:],
                        skip_group_check=True,
                    )
                nc.tensor.matmul(
                    wps[:, 128:256], ones1[:], brow_sb[:],
                    skip_group_check=True,
                )
                nc.scalar.activation(
                    bvthrow_sb[:, 0, :], wps[:, 128:256], AF.Identity
                )

            s8 = {}
            g = {}
            vg_n = [0]

            def zslice(mt, width=128):
                c = mt // 8
                o = (mt % 8) * 128
                if c == 0:
                    return zblk16_sb[:, o:o + width]
                return z8_sb[:, (c - 1) * BLK + o:(c - 1) * BLK + o + width]

            def emit_g(mt, gpool):
                gt = gpool.tile([CI, 1024], F32, tag="g", name=f"g{mt}")
                hb = 0 if mt % 2 == 0 else CO   # row-group alternation
                msl = slice(mt * 128, (mt + 1) * 128)
                for h in range(2):
                    nc.tensor.matmul(
                        gt[:, h * 512:(h + 1) * 512],
                        psi_sb[hb:hb + CO, msl],
                        phi_sb[hb:hb + CO, h * 512:(h + 1) * 512],
                    )
                g[mt] = gt

            def emit_relu(p, a, b, odd_on_act=False):
                s = sp.tile([CI, 2, 1024], FP8, tag="s", name=f"s{p}")
                nc.scalar.activation(s[:, 0, :], g.pop(a)[:], AF.Relu)
                if odd_on_act:
                    nc.scalar.activation(s[:, 1, :], g.pop(b)[:], AF.Relu)
                else:
                    nc.vector.tensor_scalar_max(s[:, 1, :], g.pop(b)[:], 0.0)
                s8[p] = s

            def emit_vg(p, a):
                s = s8.pop(p)
                vg_n[0] += 1
                for h in range(2):
                    nc.tensor.matmul(
                        vg_ps[:, h * 512:(h + 1) * 512],
                        wvth8_sb[:, a:a + 2, :],
                        s[:, :, h * 512:(h + 1) * 512],
                        start=False, stop=(vg_n[0] == MT // 2),
                        perf_mode=DR, skip_group_check=True,
                    )

            def emit_psi(c, gpool):
                base = c * 1024
                ps = gpool.tile([CI, 1024], F32, tag="g", name=f"pj{c}")
                for h in range(2):
                    if c == 0:
                        nc.tensor.matmul(
                            ps[:, h * 512:(h + 1) * 512], wpsid_sb,
                            zblk16_sb[:, h * 512:(h + 1) * 512],
                        )
                    else:
                        nc.tensor.matmul(
                            ps[:, h * 512:(h + 1) * 512], wpsid8_sb,
                            z8_sb[:, (c - 1) * BLK + h * 512:
                                  (c - 1) * BLK + (h + 1) * 512],
                        )
                # split drain across both engines: halves the PSUM
                # slot-recycle latency in the (engine-idle) front
                nc.scalar.activation(
                    psi_sb[:, base:base + 512], ps[:, 0:512],
                    AF.Identity, bias=bpsi_sb,
                )
                nc.vector.tensor_scalar_add(
                    psi_sb[:, base + 512:base + 1024], ps[:, 512:1024],
                    bpsi_sb,
                )

            def emit_phi(gpool):
                php = gpool.tile([CI, 1024], F32, tag="g", name="phj")
                for h in range(2):
                    nc.tensor.matmul(
                        php[:, h * 512:(h + 1) * 512], wphid_sb,
                        zblk16_sb[:, h * 512:(h + 1) * 512],
                    )
                nc.scalar.activation(
                    phi_sb[:, 0:512], php[:, 0:512],
                    AF.Identity, bias=bphi_sb,
                )
                nc.vector.tensor_scalar_add(
                    phi_sb[:, 512:1024], php[:, 512:1024], bphi_sb
                )

            def emit_wvth(c, gpool):
                wv_ps = gpool.tile([CI, 1024], F32, tag="g", name=f"wv{c}")
                for j in range(8):
                    mt = c * 8 + j
                    nc.tensor.matmul(
                        wv_ps[:, j * 128:(j + 1) * 128],
                        zslice(mt),
                        wtht_sb if c == 0 else wtht8_sb,
                        skip_group_check=True,
                    )
                # drain with bvth added along the free (ci) dim via the
                # partition-broadcast row. Pool cannot read PSUM and ACT
                # has no free-dim bias, so this is all DVE; the adjacent
                # pair's relu is fully taken over by ACT to compensate.
                v = wv_ps[:].rearrange("p (a b) -> p a b", a=8)
                nc.vector.tensor_tensor(
                    wvth8_sb[:, c * 8:(c + 1) * 8, :], v[:],
                    bvthrow_sb[:].broadcast_to([CI, 8, CI]),
                    ALU.add,
                )

            with tc.tile_pool(name="gps", bufs=3, space="PSUM") as gpool:
                # projections in z-arrival order; psi for all chunks first
                # (only needs z + weights), then phi.
                for c in CHUNK_ORDER:
                    emit_psi(c, gpool)
                emit_phi(gpool)
                # residual prefill: vg_ps = 4096 * z(block)
                for h in range(2):
                    nc.tensor.matmul(
                        vg_ps[:, h * 512:(h + 1) * 512],
                        ident_sb,
                        zblk16_sb[:, h * 512:(h + 1) * 512],
                        start=True, stop=False, skip_group_check=True,
                    )
                pairs = [4 * c + i for c in CHUNK_ORDER for i in range(4)]
                plan = []
                for k, p in enumerate(pairs):
                    if k % 4 == 0:
                        plan.append(("wvth", CHUNK_ORDER[k // 4]))
                    plan.append(("pair", p))
                done = []
                # first pair after each wvth: both relu tiles on ACT so
                # DVE is free for the wvth drain half
                act_takeover = {pairs[0], pairs[4], pairs[8], pairs[12]}
                for kind, val in plan:
                    if kind == "wvth":
                        emit_wvth(val, gpool)
                    else:
                        emit_g(2 * val, gpool)
                        emit_g(2 * val + 1, gpool)
                        done.append(val)
                        if len(done) >= 3:
                            emit_vg(done[-3], 2 * done[-3])
                        emit_relu(val, 2 * val, 2 * val + 1,
                                  odd_on_act=val in act_takeover)
                emit_vg(done[-2], 2 * done[-2])
                emit_vg(done[-1], 2 * done[-1])

            # ---- tail: out = vg/4096 + b_v ----
            out0 = tailp.tile([CI, 512], F32, name="out0")
            out1 = tailp.tile([CI, 512], F32, name="out1")
            nc.scalar.activation(
                out0[:], vg_ps[:, 0:512], AF.Identity, bias=bv_sb, scale=SCALE
            )
            nc.vector.tensor_scalar(
                out1[:], vg_ps[:, 512:1024], SCALE, bv_sb, ALU.mult, ALU.add
            )
            nc.sync.dma_start(out[:, 0:512], out0[:])
            nc.sync.dma_start(out[:, 512:1024], out1[:])

    nc.compile()
    return nc


def _get_nc():
    if "nc" not in _CACHE:
        _CACHE["nc"] = _build()
    return _CACHE["nc"]


def build_in_maps(z, w_theta, b_theta, w_phi, b_phi, w_psi, b_psi, w_v, b_v):
    z2 = np.asarray(z, np.float32).reshape(B, CI, N)

    wpsiT = np.asarray(w_psi, np.float32).T          # [128, 64]
    wphiT = np.asarray(w_phi, np.float32).T
    wv = np.asarray(w_v, np.float32)                 # [128, 64]
    wthT = (wv @ np.asarray(w_theta, np.float32)).T  # [128, 128]
    wpack = np.ascontiguousarray(
        np.concatenate(
            [wpsiT, wpsiT, wphiT, wphiT, wthT,
             np.float32(N) * np.eye(CI, dtype=np.float32)],
            axis=1,
        ).astype(BF16NP)
    )
    w8pack = np.ascontiguousarray(
        np.concatenate([wpsiT, wpsiT, wthT], axis=1).astype(FP8NP)
    )
    biaspack = np.stack(
        [
            np.concatenate([b_psi, b_psi]).astype(np.float32),
            np.asarray(b_v, np.float32),
            np.concatenate([b_phi, b_phi]).astype(np.float32),
            np.zeros(CI, np.float32),
        ],
        axis=1,
    ).astype(np.float32)
    bvth = wv @ np.asarray(b_theta, np.float32)      # [128]
    brow = np.ascontiguousarray(bvth.reshape(1, 128).astype(BF16NP))

    in_maps = []
    for core in range(NCORES):
        b, nb = divmod(core, NCORES // B)
        others = [c for c in range(4) if c != nb]
        zrest = np.concatenate(
            [z2[b][:, c * BLK:(c + 1) * BLK] for c in others], axis=1
        )
        in_maps.append(
            {
                "wpack": wpack,
                "w8pack": w8pack,
                "biaspack": biaspack,
                "brow": brow,
                "zblk16": np.ascontiguousarray(
                    z2[b][:, nb * BLK:(nb + 1) * BLK].astype(BF16NP)
                ),
                "z8": np.ascontiguousarray(zrest.astype(FP8NP)),
            }
        )
    return in_maps


def kernel(z, w_theta, b_theta, w_phi, b_phi, w_psi, b_psi, w_v, b_v):
    in_maps = build_in_maps(
        z, w_theta, b_theta, w_phi, b_phi, w_psi, b_psi, w_v, b_v
    )
    nc = _get_nc()
    res = run_bass_kernel_spmd(nc, in_maps, core_ids=list(range(NCORES)))

    out_full = np.empty((B, CI, N), dtype=np.float32)
    for core in range(NCORES):
        b, nb = divmod(core, NCORES // B)
        out_full[b][:, nb * BLK:(nb + 1) * BLK] = res.results[core]["out"]
    return out_full.reshape(B, CI, T, H, W)


# revision 4
# speedup vs baseline: 1.1316x; 1.0785x over previous
"""Trainium2 Bass kernel for nn_Attention (non-local-block attention).

Reference (per batch b, z flattened to [Ci=128, N=4096]):
    theta = w_theta @ z + b_theta        [64, N]
    phi   = w_phi   @ z + b_phi          [64, N]
    psi   = w_psi   @ z + b_psi          [64, N]
    G     = relu((phi^T psi) / N)        [N, N]
    out   = w_v @ (G @ theta^T)^T + b_v + z

Restructure: fold w_v into theta (host: WTH = w_v @ w_theta, bvth = w_v @
b_theta), so WVTH = WTH @ z + bvth and
    out[ci,n] = (1/N) * sum_m relu(g)[n,m] * WVTH[ci,m] + b_v[ci] + z[ci,n]

Sharding: 8 cores = 2 batches x 4 token-blocks of 1024. Chunks 1-3 of the
m (token) dim ship as fp8e4; the core's own block as bf16.

Empirical p-state model (trace-verified): the PE clock is bistable —
once dependency waits pace it at the 1.2GHz mid state it stays there;
a burst of dense back-to-back matmuls flips it to 2.4GHz. So: (a) the
PE starts on a gpsimd-memset warmup right after the preamble (no DMA
wait), (b) dense 128-col "igniter" matmuls are injected into the
about-to-be-overwritten g tile at each chunk boundary, (c) the front is
ordered so nothing stalls: psi c1/c2/c3 (fp8, early DMAs), wvth c1,
phi, psi c0, ident, then the pair loop with wvth emitted 2 pairs ahead
of its chunk. ACT's act-table load is hoisted via a dummy activation.
ACT/DVE carry only relu + half-drains each (they are the throughput
floor at ~25us); wvth bias rides K=1 ones x bvth-row prefill matmuls.
"""

import ml_dtypes
import numpy as np

import concourse.bacc as bacc
import concourse.mybir as mybir
import concourse.tile as tile
from concourse.bass_utils import run_bass_kernel_spmd

F32 = mybir.dt.float32
BF16 = mybir.dt.bfloat16
FP8 = mybir.dt.float8e4
AF = mybir.ActivationFunctionType
ALU = mybir.AluOpType
DR = mybir.MatmulPerfMode.DoubleRow
BF16NP = ml_dtypes.bfloat16
FP8NP = ml_dtypes.float8_e4m3

B, CI, CO = 2, 128, 64
T, H, W = 4, 32, 32
N = T * H * W            # 4096 tokens
NCORES = 8
BLK = N // (NCORES // B)  # 1024 tokens per core
MT = N // 128            # 32 m-tiles
SCALE = float(1.0 / N)
CHUNK_ORDER = (1, 2, 3, 0)   # z8 chunks land before zblk16
N_WARM = 28
N_IGNITE = 4

_CACHE = {}


def _build():
    nc = bacc.Bacc("TRN2", target_bir_lowering=False, debug=False)

    wpack = nc.dram_tensor("wpack", [CI, 512], BF16, kind="ExternalInput")
    w8pack = nc.dram_tensor("w8pack", [CI, 256], FP8, kind="ExternalInput")
    biaspack = nc.dram_tensor("biaspack", [CI, 4], F32, kind="ExternalInput")
    brow = nc.dram_tensor("brow", [1, 512], BF16, kind="ExternalInput")
    zblk16 = nc.dram_tensor("zblk16", [CI, BLK], BF16, kind="ExternalInput")
    z8 = nc.dram_tensor("z8", [CI, 3 * BLK], FP8, kind="ExternalInput")
    out = nc.dram_tensor("out", [CI, BLK], F32, kind="ExternalOutput")

    with tile.TileContext(nc) as tc:
        with (
            tc.tile_pool(name="const", bufs=1) as cpool,
            tc.tile_pool(name="zp", bufs=1) as zp,
            tc.tile_pool(name="proj", bufs=1) as pp,
            tc.tile_pool(name="s8p", bufs=3) as sp,
            tc.tile_pool(name="tail", bufs=1) as tailp,
            tc.tile_pool(name="vgps", bufs=1, space="PSUM") as vgpool,
        ):
            # ---- warm sources: no DMA dependency ----
            warmsrc = cpool.tile([CI, 128], BF16)
            nc.gpsimd.memset(warmsrc[:], 1.0)
            ones1 = cpool.tile([1, 128], BF16)
            nc.gpsimd.memset(ones1[:], 1.0)
            actdummy = cpool.tile([CI, 2], F32)
            nc.gpsimd.memset(actdummy[:], 0.0)

            # ---- input DMAs in need-order ----
            brow_sb = cpool.tile([1, 512], BF16)
            nc.sync.dma_start(brow_sb[:], brow[:])
            w8pack_sb = cpool.tile([CI, 256], FP8)
            nc.sync.dma_start(w8pack_sb[:], w8pack[:])
            biaspack_sb = cpool.tile([CI, 4], F32)
            nc.sync.dma_start(biaspack_sb[:], biaspack[:])
            wpack_sb = cpool.tile([CI, 512], BF16)
            nc.sync.dma_start(wpack_sb[:], wpack[:])
            zblk16_sb = zp.tile([CI, BLK], BF16)
            nc.sync.dma_start(zblk16_sb[:, 0:512], zblk16[:, 0:512])
            nc.sync.dma_start(zblk16_sb[:, 512:1024], zblk16[:, 512:1024])
            z8_sb = zp.tile([CI, 3 * BLK], FP8)
            for c in range(3):
                nc.gpsimd.dma_start(
                    z8_sb[:, c * BLK:(c + 1) * BLK],
                    z8[:, c * BLK:(c + 1) * BLK],
                )

            # ---- ACT table load hoist (1.5us otherwise paid at first
            # psi drain) ----
            actout = cpool.tile([CI, 2], F32)
            nc.scalar.activation(actout[:], actdummy[:], AF.Relu)

            wpsid_sb = wpack_sb[:, 0:128]       # [wpsiT | wpsiT]
            wphid_sb = wpack_sb[:, 128:256]     # [wphiT | wphiT]
            wtht_sb = wpack_sb[:, 256:384]      # (w_v @ w_theta).T bf16
            ident_sb = wpack_sb[:, 384:512]     # 4096 * I
            wpsid8_sb = w8pack_sb[:, 0:128]
            wtht8_sb = w8pack_sb[:, 128:256]
            bpsi_sb = biaspack_sb[:, 0:1]       # [b_psi; b_psi]
            bphi_sb = biaspack_sb[:, 2:3]       # [b_phi; b_phi]
            bv_sb = biaspack_sb[:, 1:2]

            psi_sb = pp.tile([CI, N], BF16)     # row-duplicated
            phi_sb = pp.tile([CI, BLK], BF16)   # row-duplicated
            wvth8_sb = pp.tile([CI, MT, CI], FP8)

            # vg accumulator: 2 PSUM banks, lives for the whole kernel
            vg_ps = vgpool.tile([CI, BLK], F32)

            # ---- warmup: PE busy from the moment the preamble ends, so
            # the clock ramps while the first z chunks are in flight ----
            with tc.tile_pool(name="warm", bufs=1, space="PSUM") as wpool:
                wps = wpool.tile([CI, 512], F32)
                for _ in range(N_WARM):
                    nc.tensor.matmul(
                        wps[:, 0:128], warmsrc[:], warmsrc[:],
                        skip_group_check=True,
                    )

            s8 = {}
            g = {}
            vg_n = [0]

            def zslice(mt, width=128):
                c = mt // 8
                o = (mt % 8) * 128
                if c == 0:
                    return zblk16_sb[:, o:o + width]
                return z8_sb[:, (c - 1) * BLK + o:(c - 1) * BLK + o + width]

            def emit_g(mt, gpool, ignite=False):
                gt = gpool.tile([CI, 1024], F32, tag="g", name=f"g{mt}")
                # dense 128-col bursts into the region the real matmul
                # start=True-overwrites: flips the PE p-state to full
                # after boundary turbulence.
                if ignite:
                    for _ in range(N_IGNITE):
                        nc.tensor.matmul(
                            gt[:, 0:128], warmsrc[:], warmsrc[:],
                            skip_group_check=True,
                        )
                hb = 0 if mt % 2 == 0 else CO   # row-group alternation
                msl = slice(mt * 128, (mt + 1) * 128)
                for h in range(2):
                    nc.tensor.matmul(
                        gt[:, h * 512:(h + 1) * 512],
                        psi_sb[hb:hb + CO, msl],
                        phi_sb[hb:hb + CO, h * 512:(h + 1) * 512],
                    )
                g[mt] = gt

            def emit_relu(p, a, b):
                s = sp.tile([CI, 2, 1024], FP8, tag="s", name=f"s{p}")
                nc.scalar.activation(s[:, 0, :], g.pop(a)[:], AF.Relu)
                nc.vector.tensor_scalar_max(s[:, 1, :], g.pop(b)[:], 0.0)
                s8[p] = s

            def emit_vg(p, a):
                s = s8.pop(p)
                vg_n[0] += 1
                for h in range(2):
                    nc.tensor.matmul(
                        vg_ps[:, h * 512:(h + 1) * 512],
                        wvth8_sb[:, a:a + 2, :],
                        s[:, :, h * 512:(h + 1) * 512],
                        start=False, stop=(vg_n[0] == MT // 2),
                        perf_mode=DR, skip_group_check=True,
                    )

            def emit_psi(c, gpool):
                base = c * 1024
                ps = gpool.tile([CI, 1024], F32, tag="g", name=f"pj{c}")
                for h in range(2):
                    if c == 0:
                        nc.tensor.matmul(
                            ps[:, h * 512:(h + 1) * 512], wpsid_sb,
                            zblk16_sb[:, h * 512:(h + 1) * 512],
                        )
                    else:
                        nc.tensor.matmul(
                            ps[:, h * 512:(h + 1) * 512], wpsid8_sb,
                            z8_sb[:, (c - 1) * BLK + h * 512:
                                  (c - 1) * BLK + (h + 1) * 512],
                        )
                # split drain across both engines: halves the PSUM
                # slot-recycle latency in the (engine-idle) front
                nc.scalar.activation(
                    psi_sb[:, base:base + 512], ps[:, 0:512],
                    AF.Identity, bias=bpsi_sb,
                )
                nc.vector.tensor_scalar_add(
                    psi_sb[:, base + 512:base + 1024], ps[:, 512:1024],
                    bpsi_sb,
                )

            def emit_phi(gpool):
                php = gpool.tile([CI, 1024], F32, tag="g", name="phj")
                for h in range(2):
                    nc.tensor.matmul(
                        php[:, h * 512:(h + 1) * 512], wphid_sb,
                        zblk16_sb[:, h * 512:(h + 1) * 512],
                    )
                nc.scalar.activation(
                    phi_sb[:, 0:512], php[:, 0:512],
                    AF.Identity, bias=bphi_sb,
                )
                nc.vector.tensor_scalar_add(
                    phi_sb[:, 512:1024], php[:, 512:1024], bphi_sb
                )

            def emit_wvth(c, gpool):
                wv_ps = gpool.tile([CI, 1024], F32, tag="g", name=f"wv{c}")
                # bvth bias along the free (ci) dim via K=1 ones x
                # bvth-row prefill, then z-stationary matmuls accumulate
                for h in range(2):
                    nc.tensor.matmul(
                        wv_ps[:, h * 512:(h + 1) * 512],
                        ones1[:], brow_sb[:, 0:512],
                        start=True, stop=False, skip_group_check=True,
                    )
                for j in range(8):
                    mt = c * 8 + j
                    nc.tensor.matmul(
                        wv_ps[:, j * 128:(j + 1) * 128],
                        zslice(mt),
                        wtht_sb if c == 0 else wtht8_sb,
                        start=False, stop=(j % 4 == 3),
                        skip_group_check=True,
                    )
                # balanced half-drains (pure copies, bias already in)
                nc.scalar.activation(
                    wvth8_sb[:, c * 8:c * 8 + 4, :], wv_ps[:, 0:512],
                    AF.Copy,
                )
                nc.vector.tensor_copy(
                    wvth8_sb[:, c * 8 + 4:(c + 1) * 8, :],
                    wv_ps[:, 512:1024],
                )

            with tc.tile_pool(name="gps", bufs=3, space="PSUM") as gpool:
                # front, in data-arrival order. wvth c1 before phi keeps
                # the PE busy while wpack/zblk16 land; phi before psi c0
                # because every g matmul needs phi.
                emit_psi(1, gpool)
                emit_psi(2, gpool)
                emit_psi(3, gpool)
                emit_wvth(1, gpool)
                emit_phi(gpool)
                emit_psi(0, gpool)
                # residual prefill: vg_ps = 4096 * z(block)
                for h in range(2):
                    nc.tensor.matmul(
                        vg_ps[:, h * 512:(h + 1) * 512],
                        ident_sb,
                        zblk16_sb[:, h * 512:(h + 1) * 512],
                        start=True, stop=False, skip_group_check=True,
                    )
                pairs = [4 * c + i for c in CHUNK_ORDER for i in range(4)]
                # wvth for the next chunk lands 2 pairs before that
                # chunk's pairs begin, so its ACT/DVE half-drains clear
                # the PSUM slot before the ring wraps to it.
                plan = []
                for k, p in enumerate(pairs):
                    plan.append(("pair", p))
                    if k in (1, 5, 9):
                        plan.append(("wvth", CHUNK_ORDER[k // 4 + 1]))
                chunk_first = {pairs[0], pairs[4], pairs[8], pairs[12]}
                done = []
                for kind, val in plan:
                    if kind == "wvth":
                        emit_wvth(val, gpool)
                    else:
                        emit_g(2 * val, gpool, ignite=val in chunk_first)
                        emit_g(2 * val + 1, gpool)
                        done.append(val)
                        if len(done) >= 3:
                            emit_vg(done[-3], 2 * done[-3])
                        emit_relu(val, 2 * val, 2 * val + 1)
                emit_vg(done[-2], 2 * done[-2])
                emit_vg(done[-1], 2 * done[-1])

            # ---- tail: out = vg/4096 + b_v ----
            out0 = tailp.tile([CI, 512], F32, name="out0")
            out1 = tailp.tile([CI, 512], F32, name="out1")
            nc.scalar.activation(
                out0[:], vg_ps[:, 0:512], AF.Identity, bias=bv_sb, scale=SCALE
            )
            nc.vector.tensor_scalar(
                out1[:], vg_ps[:, 512:1024], SCALE, bv_sb, ALU.mult, ALU.add
            )
            nc.sync.dma_start(out[:, 0:512], out0[:])
            nc.sync.dma_start(out[:, 512:1024], out1[:])

    nc.compile()
    return nc


def _get_nc():
    if "nc" not in _CACHE:
        _CACHE["nc"] = _build()
    return _CACHE["nc"]


def build_in_maps(z, w_theta, b_theta, w_phi, b_phi, w_psi, b_psi, w_v, b_v):
    z2 = np.asarray(z, np.float32).reshape(B, CI, N)

    wpsiT = np.asarray(w_psi, np.float32).T          # [128, 64]
    wphiT = np.asarray(w_phi, np.float32).T
    wv = np.asarray(w_v, np.float32)                 # [128, 64]
    wthT = (wv @ np.asarray(w_theta, np.float32)).T  # [128, 128]
    wpack = np.ascontiguousarray(
        np.concatenate(
            [wpsiT, wpsiT, wphiT, wphiT, wthT,
             np.float32(N) * np.eye(CI, dtype=np.float32)],
            axis=1,
        ).astype(BF16NP)
    )
    w8pack = np.ascontiguousarray(
        np.concatenate([wpsiT, wpsiT, wthT], axis=1).astype(FP8NP)
    )
    biaspack = np.stack(
        [
            np.concatenate([b_psi, b_psi]).astype(np.float32),
            np.asarray(b_v, np.float32),
            np.concatenate([b_phi, b_phi]).astype(np.float32),
            np.zeros(CI, np.float32),
        ],
        axis=1,
    ).astype(np.float32)
    bvth = wv @ np.asarray(b_theta, np.float32)      # [128]
    brow = np.ascontiguousarray(
        np.tile(bvth, 4).reshape(1, 512).astype(BF16NP)
    )

    in_maps = []
    for core in range(NCORES):
        b, nb = divmod(core, NCORES // B)
        others = [c for c in range(4) if c != nb]
        zrest = np.concatenate(
            [z2[b][:, c * BLK:(c + 1) * BLK] for c in others], axis=1
        )
        in_maps.append(
            {
                "wpack": wpack,
                "w8pack": w8pack,
                "biaspack": biaspack,
                "brow": brow,
                "zblk16": np.ascontiguousarray(
                    z2[b][:, nb * BLK:(nb + 1) * BLK].astype(BF16NP)
                ),
                "z8": np.ascontiguousarray(zrest.astype(FP8NP)),
            }
        )
    return in_maps


def kernel(z, w_theta, b_theta, w_phi, b_phi, w_psi, b_psi, w_v, b_v):
    in_maps = build_in_maps(
        z, w_theta, b_theta, w_phi, b_phi, w_psi, b_psi, w_v, b_v
    )
    nc = _get_nc()
    res = run_bass_kernel_spmd(nc, in_maps, core_ids=list(range(NCORES)))

    out_full = np.empty((B, CI, N), dtype=np.float32)
    for core in range(NCORES):
        b, nb = divmod(core, NCORES // B)
        out_full[b][:, nb * BLK:(nb + 1) * BLK] = res.results[core]["out"]
    return out_full.reshape(B, CI, T, H, W)
